# revision 37
# baseline (speedup 1.0000x reference)
"""Trainium2 Bass kernel for nn_MultiHeadAttention_KT (causal linear attention).

Math (per batch b):
  q' = leaky((q*qm) @ Wq + bq); k' = leaky((k*km) @ Wk + bk); v' = (v*vm) @ Wv
  per head h (DEPTH=64):   S_t = sum_{s<=t} k_s v_s^T ; z_t = sum_{s<=t} k_s
                           attn_t = (q_t @ S_t) / (q_t . z_t)
  out = concat_heads(attn) @ Wo + bo
Sharding: 8 cores = 2 batches x 4 head-groups (4 heads / 256 cols each).
Host transposes + bf16-casts inputs (xq = (q*qm)^T etc.); host sums the 4
partial output projections per batch (po is this core's heads' Wo slice).

All matmul operands are bf16 (1 cyc/row on PE + fast weight load); PSUM
accumulation stays f32.  Chunked linear attention (chunk C=128):
  AT   = K Q^T (per chunk, [s,t] layout)      masked with triu (s<=t)
  num  = ATm^T V_aug + Q S_aug                (V_aug = [V | 1], S_aug = [S | z])
  attn = num[:, :64] * (1/num[:, 64])
  S_aug += K_chunk^T V_aug                    (delta matmul; f32 master state
                                               on DVE + bf16 mirror)

Schedule: the 512-col projection / output-projection matmul groups are
pumped one-at-a-time between attention-chunk stages so the PE array duty
stays high in every HAM window (keeps the 2.4 GHz clock gate open), DMA
overlaps compute, and the store tail is hidden.
"""

import os
import sys

sys.path.insert(0, "/opt/trn_rl_repo")

import ml_dtypes
import numpy as np

BF16 = np.dtype(ml_dtypes.bfloat16)

B, S, D, H = 2, 2048, 1024, 16
DEPTH = 64
N_CORES = 8
HPC = 4                 # heads per core
JS = HPC * DEPTH        # 256 projected columns per core
C = 128                 # attention chunk length
NCH = S // C            # 16 chunks
IB = D // 128           # 8 contraction blocks
SCH = 512               # projection s-chunk
NSC = S // SCH          # 4 projection chunks
JAUG = DEPTH + 1        # 65 (V augmented with ones column)
OSC = 512               # output projection s-chunk
CPO = OSC // C          # attention chunks per output chunk

MM_DTYPE = "bf16"       # informational (printed by test harness)
SIM_NO_PRELU = os.environ.get("KT_SIM_NO_PRELU") == "1"  # CoreSim lacks Prelu
TRACE = False           # set True from test harness to capture NTFF profile
TRACE_CORES = None
LAST_RESULTS = None     # BassKernelResults of the last kernel() call

_PROG = None


def _build():
    import concourse.bacc as bacc
    import concourse.mybir as mybir
    import concourse.tile as tile

    dt = mybir.dt
    f32 = dt.float32
    bf = dt.bfloat16
    AF = mybir.ActivationFunctionType
    Alu = mybir.AluOpType

    nc = bacc.Bacc("TRN2", target_bir_lowering=False, debug=False,
                   num_devices=N_CORES)

    # host pre-tiles x and weights so every DMA is per-partition contiguous
    xq = nc.dram_tensor("xq", [NSC, 128, IB, SCH], bf, kind="ExternalInput").ap()
    xk = nc.dram_tensor("xk", [NSC, 128, IB, SCH], bf, kind="ExternalInput").ap()
    xv = nc.dram_tensor("xv", [NSC, 128, IB, SCH], bf, kind="ExternalInput").ap()
    wq = nc.dram_tensor("wq", [128, IB, JS], bf, kind="ExternalInput").ap()
    wk = nc.dram_tensor("wk", [128, IB, JS], bf, kind="ExternalInput").ap()
    wv = nc.dram_tensor("wv", [128, IB, JS], bf, kind="ExternalInput").ap()
    wo = nc.dram_tensor("wo", [128, 2, D], bf, kind="ExternalInput").ap()
    bqd = nc.dram_tensor("bq", [2, 128], f32, kind="ExternalInput").ap()
    bkd = nc.dram_tensor("bk", [2, 128], f32, kind="ExternalInput").ap()
    triu2 = nc.dram_tensor("triu2", [128, 256], f32, kind="ExternalInput").ap()
    ident = nc.dram_tensor("ident", [128, 128], bf, kind="ExternalInput").ap()
    po = nc.dram_tensor("po", [D, S], bf, kind="ExternalOutput").ap()

    def mm(out, lhsT, rhs, **kw):
        nc.tensor.matmul(out, lhsT, rhs, **kw)

    with tile.TileContext(nc) as tc:
        with (
            tc.tile_pool(name="persist", bufs=1) as pp,
            tc.tile_pool(name="xin", bufs=3) as xpool,
            tc.tile_pool(name="work", bufs=4) as wk_pool,
            tc.tile_pool(name="outp", bufs=6) as opool,
            tc.tile_pool(name="psA", bufs=3, space="PSUM") as psA,
            tc.tile_pool(name="psB", bufs=5, space="PSUM") as psB,
        ):
            # ---- persistent tiles -------------------------------------------
            wq_sb = pp.tile([128, IB, JS], bf, tag="wq", name="wq_sb")
            wk_sb = pp.tile([128, IB, JS], bf, tag="wk", name="wk_sb")
            wv_sb = pp.tile([128, IB, JS], bf, tag="wv", name="wv_sb")
            wo_sb = pp.tile([128, 2, D], bf, tag="wo", name="wo_sb")
            bq_sb = pp.tile([128, 2], f32, tag="bq", name="bq_sb")
            bk_sb = pp.tile([128, 2], f32, tag="bk", name="bk_sb")
            triu_sb = pp.tile([128, 256], f32, tag="triu", name="triu_sb")
            ident_sb = pp.tile([128, 128], bf, tag="ident", name="ident_sb")

            qT_sb = [pp.tile([128, S], bf, tag=f"qT{jb}", name=f"qT{jb}") for jb in range(2)]
            kT_sb = [pp.tile([128, S], bf, tag=f"kT{jb}", name=f"kT{jb}") for jb in range(2)]
            aT_c = pp.tile([128, 2, S], bf, tag="aTc", name="aTc")
            vaug_sb = [pp.tile([128, HPC * JAUG], bf, tag=f"vaug{i}", name=f"vaug{i}")
                       for i in range(NCH)]
            # two heads per tile: head h at partitions (h%2)*64 .. +64
            saug_sb = [pp.tile([128, JAUG], f32, tag=f"saug{jb}", name=f"saug{jb}")
                       for jb in range(2)]
            saug_bf = [pp.tile([128, JAUG], bf, tag=f"saugb{jb}", name=f"saugb{jb}")
                       for jb in range(2)]
            attn2_sb = [pp.tile([128, 2 * DEPTH], bf, tag=f"attn2{jb}", name=f"attn2{jb}")
                        for jb in range(2)]

            # ---- initial loads: q path first so compute starts ASAP ---------
            x_tiles = {}

            def load_x(sc):
                xq_t = xpool.tile([128, IB, SCH], bf, tag="xq")
                xk_t = xpool.tile([128, IB, SCH], bf, tag="xk")
                xv_t = xpool.tile([128, IB, SCH], bf, tag="xv")
                nc.sync.dma_start(xq_t[:], xq[sc])
                nc.scalar.dma_start(xk_t[:], xk[sc])
                nc.sync.dma_start(xv_t[:], xv[sc])
                x_tiles[sc] = (xq_t, xk_t, xv_t)

            # chunk-0 inputs arrive in 256-col halves so the first projection
            # matmuls can start ~4us earlier (the PE is DMA-starved here, and
            # early sustained matmuls also open the HAM clock gate sooner)
            HA = SCH // 2
            xq_t0 = xpool.tile([128, IB, SCH], bf, tag="xq")
            xk_t0 = xpool.tile([128, IB, SCH], bf, tag="xk")
            xv_t0 = xpool.tile([128, IB, SCH], bf, tag="xv")
            nc.sync.dma_start(xq_t0[:, :, 0:HA], xq[0][:, :, 0:HA])
            nc.scalar.dma_start(wq_sb[:], wq)
            nc.scalar.dma_start(bq_sb[:], bqd.rearrange("jb p -> p jb"))
            nc.scalar.dma_start(bk_sb[:], bkd.rearrange("jb p -> p jb"))
            nc.sync.dma_start(xq_t0[:, :, HA:SCH], xq[0][:, :, HA:SCH])
            nc.scalar.dma_start(xk_t0[:, :, 0:HA], xk[0][:, :, 0:HA])
            nc.scalar.dma_start(wk_sb[:], wk)
            nc.sync.dma_start(xv_t0[:, :, 0:HA], xv[0][:, :, 0:HA])
            nc.scalar.dma_start(xk_t0[:, :, HA:SCH], xk[0][:, :, HA:SCH])
            nc.sync.dma_start(xv_t0[:, :, HA:SCH], xv[0][:, :, HA:SCH])
            nc.sync.dma_start(wv_sb[:], wv)
            nc.scalar.dma_start(triu_sb[:], triu2)
            nc.scalar.dma_start(ident_sb[:], ident)
            nc.scalar.dma_start(wo_sb[:], wo)
            x_tiles[0] = (xq_t0, xk_t0, xv_t0)
            load_x(1)

            # ---- pumpable work units (one PSUM group each) ------------------
            def unit_qk(which, sc, jb):
                s0 = sc * SCH
                x_t = x_tiles[sc][0 if which == "q" else 1]
                w_sb = wq_sb if which == "q" else wk_sb
                b_sb = bq_sb if which == "q" else bk_sb
                dst = qT_sb if which == "q" else kT_sb
                ps = psA.tile([128, SCH], f32, tag="A")
                for ib in range(IB):
                    mm(ps[:], w_sb[:, ib, jb * 128:(jb + 1) * 128],
                       x_t[:, ib, :],
                       start=(ib == 0), stop=(ib == IB - 1))
                nc.scalar.activation(
                    dst[jb][:, s0:s0 + SCH], ps[:],
                    AF.Identity if SIM_NO_PRELU else AF.Prelu,
                    bias=b_sb[:, jb:jb + 1], scale=1.0, alpha=0.1)

            def unit_v(sc, ss):
                x_t = x_tiles[sc][2]
                ps = psA.tile([128, JS], f32, tag="A")
                for ib in range(IB):
                    mm(ps[:], x_t[:, ib, ss * 128:(ss + 1) * 128],
                       wv_sb[:, ib, :],
                       start=(ib == 0), stop=(ib == IB - 1))
                vt = vaug_sb[sc * (SCH // 128) + ss]
                vt_r = vt[:].rearrange("p (h e) -> p h e", h=HPC)
                nc.scalar.activation(
                    vt_r[:, :, 0:DEPTH],
                    ps[:].rearrange("p (h e) -> p h e", h=HPC), AF.Copy)
                nc.vector.memset(vt_r[:, :, DEPTH:JAUG], 1.0)

            def unit_qk_half(which, sc, jb, half):
                s0 = sc * SCH + half * (SCH // 2)
                c0 = half * (SCH // 2)
                x_t = x_tiles[sc][0 if which == "q" else 1]
                w_sb = wq_sb if which == "q" else wk_sb
                b_sb = bq_sb if which == "q" else bk_sb
                dst = qT_sb if which == "q" else kT_sb
                ps = psA.tile([128, SCH // 2], f32, tag="A")
                for ib in range(IB):
                    mm(ps[:], w_sb[:, ib, jb * 128:(jb + 1) * 128],
                       x_t[:, ib, c0:c0 + SCH // 2],
                       start=(ib == 0), stop=(ib == IB - 1))
                nc.scalar.activation(
                    dst[jb][:, s0:s0 + SCH // 2], ps[:],
                    AF.Identity if SIM_NO_PRELU else AF.Prelu,
                    bias=b_sb[:, jb:jb + 1], scale=1.0, alpha=0.1)

            def proj_units(sc):
                u = []
                for jb in range(2):
                    u.append(lambda jb=jb: unit_qk("q", sc, jb))
                    u.append(lambda jb=jb: unit_qk("k", sc, jb))
                for ss in range(SCH // 128):
                    u.append(lambda ss=ss: unit_v(sc, ss))
                return u

            def proj_units0():
                # chunk-0 variant: q/k at half-chunk granularity, ordered to
                # match the staggered arrival of the split input DMAs
                u = []
                for half in range(2):
                    for which in ("q", "k"):
                        for jb in range(2):
                            u.append(lambda w=which, jb=jb, h=half:
                                     unit_qk_half(w, 0, jb, h))
                for ss in range(SCH // 128):
                    u.append(lambda ss=ss: unit_v(0, ss))
                return u

            po_r = po.rearrange("(ob p) s -> ob p s", p=128)

            def unit_p3(o0, osc, ob):
                ps = psA.tile([128, OSC], f32, tag="A")
                for jb in range(2):
                    mm(ps[:, 0:osc], wo_sb[:, jb, ob * 128:(ob + 1) * 128],
                       aT_c[:, jb, o0:o0 + osc],
                       start=(jb == 0), stop=(jb == 1))
                ot = opool.tile([128, OSC], bf, tag="ot")
                if ob % 2 == 0:
                    nc.vector.tensor_copy(ot[:, 0:osc], ps[:, 0:osc])
                else:
                    nc.scalar.activation(ot[:, 0:osc], ps[:, 0:osc], AF.Copy)
                q_eng = nc.sync if ob % 2 == 0 else nc.scalar
                q_eng.dma_start(po_r[ob, :, o0:o0 + osc], ot[:, 0:osc])

            def p3_units(o0, osc=OSC):
                return [lambda ob=ob: unit_p3(o0, osc, ob)
                        for ob in range(D // 128)]

            pending = []

            def pump():
                if pending:
                    u = pending.pop(0)
                    if u is not None:
                        u()

            # ---- attention chunk (pumps a work unit between stages) ---------
            def chunk(ci):
                scol = ci * C
                if ci > 0:
                    for jb in range(2):
                        nc.vector.tensor_copy(saug_bf[jb][:], saug_sb[jb][:])

                # stage 1: K transposes (both heads in one op) + scores
                knats = []
                atm = []
                for jb in range(2):
                    knat_ps = psB.tile([128, 2 * DEPTH], bf, tag="B")
                    nc.tensor.transpose(knat_ps[:],
                                        kT_sb[jb][:, scol:scol + C],
                                        ident_sb[:])
                    knat = wk_pool.tile([128, 2 * DEPTH], bf, tag="knat")
                    nc.vector.tensor_copy(knat[:], knat_ps[:])
                    knats.append(knat)
                    am = wk_pool.tile([128, 2 * C], bf, tag="atm")
                    for hh in range(2):
                        jo = hh * DEPTH
                        at = psA.tile([128, C], f32, tag="A")
                        mm(at[:], kT_sb[jb][jo:jo + DEPTH, scol:scol + C],
                           qT_sb[jb][jo:jo + DEPTH, scol:scol + C],
                           start=True, stop=True)
                        nc.vector.tensor_tensor(am[:, hh * C:(hh + 1) * C],
                                                at[:], triu_sb[:, 0:C],
                                                op=Alu.mult)
                    atm.append(am)
                pump()

                # stage 2: numerators + attn, per jb
                for jb in range(2):
                    for hh in range(2):
                        jo = hh * DEPTH
                        h = jb * 2 + hh
                        vt = vaug_sb[ci][:, h * JAUG:(h + 1) * JAUG]
                        nump = psB.tile([128, JAUG], f32, tag="B")
                        mm(nump[:], atm[jb][:, hh * C:(hh + 1) * C], vt,
                           start=True, stop=(ci == 0))
                        if ci > 0:
                            mm(nump[:], qT_sb[jb][jo:jo + DEPTH, scol:scol + C],
                               saug_bf[jb][jo:jo + DEPTH, :],
                               start=False, stop=True)
                        recip = wk_pool.tile([128, 1], f32, tag="recip")
                        nc.vector.reciprocal(recip[:], nump[:, DEPTH:JAUG])
                        dstap = attn2_sb[jb][:, hh * DEPTH:(hh + 1) * DEPTH]
                        if jb == 0:
                            nc.vector.tensor_scalar_mul(
                                dstap, nump[:, 0:DEPTH], recip[:])
                        else:
                            nc.scalar.activation(dstap, nump[:, 0:DEPTH],
                                                 AF.Copy, scale=recip[:])
                    pump()

                # stage 3: state update S_aug += K^T V_aug
                if ci < NCH - 1:
                    for jb in range(2):
                        d_ps = psB.tile([128, JAUG], f32, tag="B")
                        for hh in range(2):
                            jo = hh * DEPTH
                            h = jb * 2 + hh
                            vt = vaug_sb[ci][:, h * JAUG:(h + 1) * JAUG]
                            mm(d_ps[jo:jo + DEPTH, :],
                               knats[jb][:, jo:jo + DEPTH],
                               vt, start=True, stop=True)
                        if ci == 0:
                            nc.vector.tensor_copy(saug_sb[jb][:], d_ps[:])
                        else:
                            nc.vector.tensor_add(saug_sb[jb][:],
                                                 saug_sb[jb][:], d_ps[:])

                # stage 4: transpose attn -> aT columns
                for jb in range(2):
                    at2_ps = psB.tile([128, C], bf, tag="B")
                    nc.tensor.transpose(at2_ps[:], attn2_sb[jb][:], ident_sb[:])
                    nc.scalar.activation(aT_c[:, jb, scol:scol + C],
                                         at2_ps[:], AF.Copy)
                pump()

            # ---- schedule ---------------------------------------------------
            # Output-projection units for columns finished by earlier groups
            # are spread evenly over groups 1-3 (14/14/12 with the projection
            # units) so no group's attention chunks run undiluted and
            # re-throttle the PE clock; the last 512 columns run as one
            # dense block after the final chunk.
            p3_pool = p3_units(0) + p3_units(OSC) + p3_units(2 * OSC)
            p3_alloc = {1: p3_pool[0:6], 2: p3_pool[6:12], 3: p3_pool[12:24]}
            for u in proj_units0():
                u()
            for g in range(NSC):
                if g + 2 < NSC:
                    load_x(g + 2)
                pending = []
                a = proj_units(g + 1) if g + 1 < NSC else []
                b = list(p3_alloc.get(g, []))
                if a:
                    while a or b:
                        if a:
                            pending.append(a.pop(0))
                        if b:
                            pending.append(b.pop(0))
                else:
                    # no projection units left: three p3 units per chunk
                    while b:
                        pending.extend(b[0:3])
                        pending.append(None)
                        b = b[3:]
                for t in range(CPO):
                    chunk(CPO * g + t)
                while pending:
                    pump()
            # tail: the last group's full 512 output columns as one dense
            # block of 512-col matmuls (back-to-back keeps the clock gate
            # open; copies and stores trail on vector/scalar + both queues)
            for u in p3_units((NSC - 1) * OSC):
                u()

    nc.compile()
    return nc


def _get_prog():
    global _PROG
    if _PROG is None:
        _PROG = _build()
    return _PROG


def kernel(q, k, v, query_mask, key_mask, value_mask,
           Wq, bq, Wk, bk, Wv, bv, Wo, bo):
    global LAST_RESULTS
    from concourse import bass_utils

    q = np.asarray(q, np.float32)
    k = np.asarray(k, np.float32)
    v = np.asarray(v, np.float32)
    qm = q * np.asarray(query_mask, np.float32)
    km = k * np.asarray(key_mask, np.float32)
    vm = v * np.asarray(value_mask, np.float32)
    Wq = np.asarray(Wq, np.float32)
    Wk = np.asarray(Wk, np.float32)
    Wv = np.asarray(Wv, np.float32)
    Wo = np.asarray(Wo, np.float32)
    bq = np.asarray(bq, np.float32)
    bk = np.asarray(bk, np.float32)
    bv = np.asarray(bv, np.float32)
    bo = np.asarray(bo, np.float32)
    assert not np.any(bv), "kernel assumes bv == 0 (true for this problem)"

    nc = _get_prog()

    triu1 = np.triu(np.ones((128, 128), np.float32))
    triu2 = np.concatenate([triu1, triu1], axis=1)
    ident = np.eye(128, dtype=np.float32).astype(BF16)

    def tile_x(a):  # a: [S, D] -> [NSC, 128, IB, SCH], per-partition contiguous
        return a.reshape(NSC, SCH, IB, 128).transpose(0, 3, 2, 1).astype(BF16)

    def tile_w(w):  # w: [D, JS] -> [128, IB, JS]
        return w.reshape(IB, 128, JS).transpose(1, 0, 2).astype(BF16)

    xqs = [tile_x(qm[b]) for b in range(B)]
    xks = [tile_x(km[b]) for b in range(B)]
    xvs = [tile_x(vm[b]) for b in range(B)]

    in_maps = []
    for c in range(N_CORES):
        b, g = divmod(c, HPC)
        js = slice(g * JS, (g + 1) * JS)
        in_maps.append({
            "xq": xqs[b], "xk": xks[b], "xv": xvs[b],
            "wq": tile_w(Wq[:, js]),
            "wk": tile_w(Wk[:, js]),
            "wv": tile_w(Wv[:, js]),
            "wo": Wo[js, :].reshape(2, 128, D).transpose(1, 0, 2).astype(BF16),
            "bq": np.ascontiguousarray(bq[js].reshape(2, 128)),
            "bk": np.ascontiguousarray(bk[js].reshape(2, 128)),
            "triu2": triu2, "ident": ident,
        })

    res = bass_utils.run_bass_kernel_spmd(
        nc, in_maps, core_ids=list(range(N_CORES)),
        trace=TRACE, trace_cores=TRACE_CORES)
    LAST_RESULTS = res

    out = np.zeros((B, S, D), np.float32)
    for c in range(N_CORES):
        out[c // HPC] += res.results[c]["po"].astype(np.float32).T
    out += bo
    return out


# revision 38
# speedup vs baseline: 1.0812x; 1.0812x over previous
"""Trainium2 Bass kernel for nn_MultiHeadAttention_KT (causal linear attention).

Math (per batch b):
  q' = leaky((q*qm) @ Wq + bq); k' = leaky((k*km) @ Wk + bk); v' = (v*vm) @ Wv
  per head h (DEPTH=64):   S_t = sum_{s<=t} k_s v_s^T ; z_t = sum_{s<=t} k_s
                           attn_t = (q_t @ S_t) / (q_t . z_t)
  out = concat_heads(attn) @ Wo + bo
Sharding: 8 cores = 2 batches x 4 head-groups (4 heads / 256 cols each).
Host transposes + bf16-casts inputs (xq = (q*qm)^T etc.); host sums the 4
partial output projections per batch (po is this core's heads' Wo slice).

All matmul operands are bf16 (1 cyc/row on PE + fast weight load); PSUM
accumulation stays f32.  Chunked linear attention (chunk C=128):
  AT   = K Q^T (per chunk, [s,t] layout)      masked with triu (s<=t)
  num  = ATm^T V_aug + Q S_aug                (V_aug = [V | 1], S_aug = [S | z])
  attn = num[:, :64] * (1/num[:, 64])
  S_aug += K_chunk^T V_aug                    (delta matmul; f32 master state
                                               on DVE + bf16 mirror)

Schedule: the 512-col projection / output-projection matmul groups are
pumped one-at-a-time between attention-chunk stages so the PE array duty
stays high in every HAM window (keeps the 2.4 GHz clock gate open), DMA
overlaps compute, and the store tail is hidden.
"""

import os
import sys

sys.path.insert(0, "/opt/trn_rl_repo")

import ml_dtypes
import numpy as np

BF16 = np.dtype(ml_dtypes.bfloat16)

B, S, D, H = 2, 2048, 1024, 16
DEPTH = 64
N_CORES = 8
HPC = 4                 # heads per core
JS = HPC * DEPTH        # 256 projected columns per core
C = 128                 # attention chunk length
NCH = S // C            # 16 chunks
IB = D // 128           # 8 contraction blocks
SCH = 512               # projection s-chunk
NSC = S // SCH          # 4 projection chunks
JAUG = DEPTH + 1        # 65 (V augmented with ones column)
OSC = 512               # output projection s-chunk
CPO = OSC // C          # attention chunks per output chunk

MM_DTYPE = "bf16"       # informational (printed by test harness)
SIM_NO_PRELU = os.environ.get("KT_SIM_NO_PRELU") == "1"  # CoreSim lacks Prelu
TRACE = False           # set True from test harness to capture NTFF profile
TRACE_CORES = None
LAST_RESULTS = None     # BassKernelResults of the last kernel() call

_PROG = None


def _build():
    import concourse.bacc as bacc
    import concourse.mybir as mybir
    import concourse.tile as tile

    dt = mybir.dt
    f32 = dt.float32
    bf = dt.bfloat16
    AF = mybir.ActivationFunctionType
    Alu = mybir.AluOpType

    nc = bacc.Bacc("TRN2", target_bir_lowering=False, debug=False,
                   num_devices=N_CORES)

    # host pre-tiles x and weights so every DMA is per-partition contiguous
    xq = nc.dram_tensor("xq", [NSC, 128, IB, SCH], bf, kind="ExternalInput").ap()
    xk = nc.dram_tensor("xk", [NSC, 128, IB, SCH], bf, kind="ExternalInput").ap()
    xv = nc.dram_tensor("xv", [NSC, 128, IB, SCH], bf, kind="ExternalInput").ap()
    wq = nc.dram_tensor("wq", [128, IB, JS], bf, kind="ExternalInput").ap()
    wk = nc.dram_tensor("wk", [128, IB, JS], bf, kind="ExternalInput").ap()
    wv = nc.dram_tensor("wv", [128, IB, JS], bf, kind="ExternalInput").ap()
    wo = nc.dram_tensor("wo", [128, 2, D], bf, kind="ExternalInput").ap()
    bqd = nc.dram_tensor("bq", [2, 128], f32, kind="ExternalInput").ap()
    bkd = nc.dram_tensor("bk", [2, 128], f32, kind="ExternalInput").ap()
    triu2 = nc.dram_tensor("triu2", [128, 256], f32, kind="ExternalInput").ap()
    ident = nc.dram_tensor("ident", [128, 128], bf, kind="ExternalInput").ap()
    po = nc.dram_tensor("po", [D, S], bf, kind="ExternalOutput").ap()

    def mm(out, lhsT, rhs, **kw):
        nc.tensor.matmul(out, lhsT, rhs, **kw)

    with tile.TileContext(nc) as tc:
        with (
            tc.tile_pool(name="persist", bufs=1) as pp,
            tc.tile_pool(name="xin", bufs=3) as xpool,
            tc.tile_pool(name="work", bufs=4) as wk_pool,
            tc.tile_pool(name="outp", bufs=6) as opool,
            tc.tile_pool(name="psA", bufs=3, space="PSUM") as psA,
            tc.tile_pool(name="psB", bufs=5, space="PSUM") as psB,
        ):
            # ---- persistent tiles -------------------------------------------
            wq_sb = pp.tile([128, IB, JS], bf, tag="wq", name="wq_sb")
            wk_sb = pp.tile([128, IB, JS], bf, tag="wk", name="wk_sb")
            wv_sb = pp.tile([128, IB, JS], bf, tag="wv", name="wv_sb")
            wo_sb = pp.tile([128, 2, D], bf, tag="wo", name="wo_sb")
            bq_sb = pp.tile([128, 2], f32, tag="bq", name="bq_sb")
            bk_sb = pp.tile([128, 2], f32, tag="bk", name="bk_sb")
            triu_sb = pp.tile([128, 256], f32, tag="triu", name="triu_sb")
            ident_sb = pp.tile([128, 128], bf, tag="ident", name="ident_sb")

            qT_sb = [pp.tile([128, S], bf, tag=f"qT{jb}", name=f"qT{jb}") for jb in range(2)]
            kT_sb = [pp.tile([128, S], bf, tag=f"kT{jb}", name=f"kT{jb}") for jb in range(2)]
            aT_c = pp.tile([128, 2, S], bf, tag="aTc", name="aTc")
            vaug_sb = [pp.tile([128, HPC * JAUG], bf, tag=f"vaug{i}", name=f"vaug{i}")
                       for i in range(NCH)]
            # two heads per tile: head h at partitions (h%2)*64 .. +64
            saug_sb = [pp.tile([128, JAUG], f32, tag=f"saug{jb}", name=f"saug{jb}")
                       for jb in range(2)]
            saug_bf = [pp.tile([128, JAUG], bf, tag=f"saugb{jb}", name=f"saugb{jb}")
                       for jb in range(2)]
            attn2_sb = [pp.tile([128, 2 * DEPTH], bf, tag=f"attn2{jb}", name=f"attn2{jb}")
                        for jb in range(2)]

            # ---- initial loads: q path first so compute starts ASAP ---------
            x_tiles = {}

            def load_x(sc):
                xq_t = xpool.tile([128, IB, SCH], bf, tag="xq")
                xk_t = xpool.tile([128, IB, SCH], bf, tag="xk")
                xv_t = xpool.tile([128, IB, SCH], bf, tag="xv")
                nc.sync.dma_start(xq_t[:], xq[sc])
                nc.scalar.dma_start(xk_t[:], xk[sc])
                nc.sync.dma_start(xv_t[:], xv[sc])
                x_tiles[sc] = (xq_t, xk_t, xv_t)

            # chunk-0 inputs arrive in 256-col halves so the first projection
            # matmuls can start ~4us earlier (the PE is DMA-starved here, and
            # early sustained matmuls also open the HAM clock gate sooner)
            HA = SCH // 2
            xq_t0 = xpool.tile([128, IB, SCH], bf, tag="xq")
            xk_t0 = xpool.tile([128, IB, SCH], bf, tag="xk")
            xv_t0 = xpool.tile([128, IB, SCH], bf, tag="xv")
            nc.sync.dma_start(xq_t0[:, :, 0:HA], xq[0][:, :, 0:HA])
            nc.scalar.dma_start(wq_sb[:], wq)
            nc.scalar.dma_start(bq_sb[:], bqd.rearrange("jb p -> p jb"))
            nc.scalar.dma_start(bk_sb[:], bkd.rearrange("jb p -> p jb"))
            nc.sync.dma_start(xq_t0[:, :, HA:SCH], xq[0][:, :, HA:SCH])
            nc.scalar.dma_start(xk_t0[:, :, 0:HA], xk[0][:, :, 0:HA])
            nc.scalar.dma_start(wk_sb[:], wk)
            nc.sync.dma_start(xv_t0[:, :, 0:HA], xv[0][:, :, 0:HA])
            nc.scalar.dma_start(xk_t0[:, :, HA:SCH], xk[0][:, :, HA:SCH])
            nc.sync.dma_start(xv_t0[:, :, HA:SCH], xv[0][:, :, HA:SCH])
            nc.sync.dma_start(wv_sb[:], wv)
            nc.scalar.dma_start(triu_sb[:], triu2)
            nc.scalar.dma_start(ident_sb[:], ident)
            nc.scalar.dma_start(wo_sb[:], wo)
            x_tiles[0] = (xq_t0, xk_t0, xv_t0)
            load_x(1)

            # ---- pumpable work units (one PSUM group each) ------------------
            def unit_qk(which, sc, jb):
                s0 = sc * SCH
                x_t = x_tiles[sc][0 if which == "q" else 1]
                w_sb = wq_sb if which == "q" else wk_sb
                b_sb = bq_sb if which == "q" else bk_sb
                dst = qT_sb if which == "q" else kT_sb
                ps = psA.tile([128, SCH], f32, tag="A")
                for ib in range(IB):
                    mm(ps[:], w_sb[:, ib, jb * 128:(jb + 1) * 128],
                       x_t[:, ib, :],
                       start=(ib == 0), stop=(ib == IB - 1))
                nc.scalar.activation(
                    dst[jb][:, s0:s0 + SCH], ps[:],
                    AF.Identity if SIM_NO_PRELU else AF.Prelu,
                    bias=b_sb[:, jb:jb + 1], scale=1.0, alpha=0.1)

            def unit_v(sc, ss):
                x_t = x_tiles[sc][2]
                ps = psA.tile([128, JS], f32, tag="A")
                for ib in range(IB):
                    mm(ps[:], x_t[:, ib, ss * 128:(ss + 1) * 128],
                       wv_sb[:, ib, :],
                       start=(ib == 0), stop=(ib == IB - 1))
                vt = vaug_sb[sc * (SCH // 128) + ss]
                vt_r = vt[:].rearrange("p (h e) -> p h e", h=HPC)
                nc.scalar.activation(
                    vt_r[:, :, 0:DEPTH],
                    ps[:].rearrange("p (h e) -> p h e", h=HPC), AF.Copy)
                nc.vector.memset(vt_r[:, :, DEPTH:JAUG], 1.0)

            def unit_qk_half(which, sc, jb, half):
                s0 = sc * SCH + half * (SCH // 2)
                c0 = half * (SCH // 2)
                x_t = x_tiles[sc][0 if which == "q" else 1]
                w_sb = wq_sb if which == "q" else wk_sb
                b_sb = bq_sb if which == "q" else bk_sb
                dst = qT_sb if which == "q" else kT_sb
                ps = psA.tile([128, SCH // 2], f32, tag="A")
                for ib in range(IB):
                    mm(ps[:], w_sb[:, ib, jb * 128:(jb + 1) * 128],
                       x_t[:, ib, c0:c0 + SCH // 2],
                       start=(ib == 0), stop=(ib == IB - 1))
                nc.scalar.activation(
                    dst[jb][:, s0:s0 + SCH // 2], ps[:],
                    AF.Identity if SIM_NO_PRELU else AF.Prelu,
                    bias=b_sb[:, jb:jb + 1], scale=1.0, alpha=0.1)

            def proj_units(sc):
                u = []
                for jb in range(2):
                    u.append(lambda jb=jb: unit_qk("q", sc, jb))
                    u.append(lambda jb=jb: unit_qk("k", sc, jb))
                for ss in range(SCH // 128):
                    u.append(lambda ss=ss: unit_v(sc, ss))
                return u

            def proj_units0():
                # chunk-0 variant: q/k at half-chunk granularity, ordered to
                # match the staggered arrival of the split input DMAs
                u = []
                for half in range(2):
                    for which in ("q", "k"):
                        for jb in range(2):
                            u.append(lambda w=which, jb=jb, h=half:
                                     unit_qk_half(w, 0, jb, h))
                for ss in range(SCH // 128):
                    u.append(lambda ss=ss: unit_v(0, ss))
                return u

            po_r = po.rearrange("(ob p) s -> ob p s", p=128)

            def unit_p3(o0, osc, ob):
                ps = psA.tile([128, OSC], f32, tag="A")
                for jb in range(2):
                    mm(ps[:, 0:osc], wo_sb[:, jb, ob * 128:(ob + 1) * 128],
                       aT_c[:, jb, o0:o0 + osc],
                       start=(jb == 0), stop=(jb == 1))
                ot = opool.tile([128, OSC], bf, tag="ot")
                if ob % 2 == 0:
                    nc.vector.tensor_copy(ot[:, 0:osc], ps[:, 0:osc])
                else:
                    nc.scalar.activation(ot[:, 0:osc], ps[:, 0:osc], AF.Copy)
                q_eng = nc.sync if ob % 2 == 0 else nc.scalar
                q_eng.dma_start(po_r[ob, :, o0:o0 + osc], ot[:, 0:osc])

            def p3_units(o0, osc=OSC):
                return [lambda ob=ob: unit_p3(o0, osc, ob)
                        for ob in range(D // 128)]

            pending = []

            def pump():
                if pending:
                    u = pending.pop(0)
                    if u is not None:
                        u()

            # ---- attention chunk (pumps a work unit between stages) ---------
            def chunk(ci):
                scol = ci * C
                if ci > 0:
                    for jb in range(2):
                        nc.vector.tensor_copy(saug_bf[jb][:], saug_sb[jb][:])

                # stage 1: K transposes (both heads in one op) + scores
                knats = []
                atm = []
                for jb in range(2):
                    knat_ps = psB.tile([128, 2 * DEPTH], bf, tag="B")
                    nc.tensor.transpose(knat_ps[:],
                                        kT_sb[jb][:, scol:scol + C],
                                        ident_sb[:])
                    knat = wk_pool.tile([128, 2 * DEPTH], bf, tag="knat")
                    nc.vector.tensor_copy(knat[:], knat_ps[:])
                    knats.append(knat)
                    am = wk_pool.tile([128, 2 * C], bf, tag="atm")
                    for hh in range(2):
                        jo = hh * DEPTH
                        at = psA.tile([128, C], f32, tag="A")
                        mm(at[:], kT_sb[jb][jo:jo + DEPTH, scol:scol + C],
                           qT_sb[jb][jo:jo + DEPTH, scol:scol + C],
                           start=True, stop=True)
                        nc.vector.tensor_tensor(am[:, hh * C:(hh + 1) * C],
                                                at[:], triu_sb[:, 0:C],
                                                op=Alu.mult)
                    atm.append(am)
                pump()

                # stage 2: numerators + attn, per jb
                for jb in range(2):
                    for hh in range(2):
                        jo = hh * DEPTH
                        h = jb * 2 + hh
                        vt = vaug_sb[ci][:, h * JAUG:(h + 1) * JAUG]
                        nump = psB.tile([128, JAUG], f32, tag="B")
                        mm(nump[:], atm[jb][:, hh * C:(hh + 1) * C], vt,
                           start=True, stop=(ci == 0))
                        if ci > 0:
                            mm(nump[:], qT_sb[jb][jo:jo + DEPTH, scol:scol + C],
                               saug_bf[jb][jo:jo + DEPTH, :],
                               start=False, stop=True)
                        recip = wk_pool.tile([128, 1], f32, tag="recip")
                        nc.vector.reciprocal(recip[:], nump[:, DEPTH:JAUG])
                        dstap = attn2_sb[jb][:, hh * DEPTH:(hh + 1) * DEPTH]
                        if jb == 0:
                            nc.vector.tensor_scalar_mul(
                                dstap, nump[:, 0:DEPTH], recip[:])
                        else:
                            nc.scalar.activation(dstap, nump[:, 0:DEPTH],
                                                 AF.Copy, scale=recip[:])
                    pump()

                # stage 3: state update S_aug += K^T V_aug
                if ci < NCH - 1:
                    for jb in range(2):
                        d_ps = psB.tile([128, JAUG], f32, tag="B")
                        for hh in range(2):
                            jo = hh * DEPTH
                            h = jb * 2 + hh
                            vt = vaug_sb[ci][:, h * JAUG:(h + 1) * JAUG]
                            mm(d_ps[jo:jo + DEPTH, :],
                               knats[jb][:, jo:jo + DEPTH],
                               vt, start=True, stop=True)
                        if ci == 0:
                            nc.vector.tensor_copy(saug_sb[jb][:], d_ps[:])
                        else:
                            nc.vector.tensor_add(saug_sb[jb][:],
                                                 saug_sb[jb][:], d_ps[:])

                # stage 4: transpose attn -> aT columns
                for jb in range(2):
                    at2_ps = psB.tile([128, C], bf, tag="B")
                    nc.tensor.transpose(at2_ps[:], attn2_sb[jb][:], ident_sb[:])
                    nc.scalar.activation(aT_c[:, jb, scol:scol + C],
                                         at2_ps[:], AF.Copy)
                pump()

            # ---- schedule ---------------------------------------------------
            # Groups 1-2 need >=4 pump units per chunk to hold the PE clock
            # gate open (measured: 3.5/chunk already re-throttles), so they
            # keep the full 8 projection + 8 output-projection units; group 3
            # gets what remains and the last 512 columns run as one dense
            # block after the final chunk.
            for u in proj_units0():
                u()
            for g in range(NSC):
                if g + 2 < NSC:
                    load_x(g + 2)
                pending = []
                a = proj_units(g + 1) if g + 1 < NSC else []
                if g == NSC - 1:
                    pending = []
                    for u in p3_units((g - 1) * OSC):
                        pending.append(u)
                        pending.append(None)
                else:
                    b = p3_units((g - 1) * OSC) if g >= 1 else [None] * 8
                    while a or b:
                        if a:
                            pending.append(a.pop(0))
                        if b:
                            pending.append(b.pop(0))
                for t in range(CPO):
                    chunk(CPO * g + t)
                while pending:
                    pump()
            # tail: the last group's full 512 output columns as one dense
            # block of 512-col matmuls (back-to-back keeps the clock gate
            # open; copies and stores trail on vector/scalar + both queues)
            for u in p3_units((NSC - 1) * OSC):
                u()

    nc.compile()
    return nc


def _get_prog():
    global _PROG
    if _PROG is None:
        _PROG = _build()
    return _PROG


def kernel(q, k, v, query_mask, key_mask, value_mask,
           Wq, bq, Wk, bk, Wv, bv, Wo, bo):
    global LAST_RESULTS
    from concourse import bass_utils

    q = np.asarray(q, np.float32)
    k = np.asarray(k, np.float32)
    v = np.asarray(v, np.float32)
    qm = q * np.asarray(query_mask, np.float32)
    km = k * np.asarray(key_mask, np.float32)
    vm = v * np.asarray(value_mask, np.float32)
    Wq = np.asarray(Wq, np.float32)
    Wk = np.asarray(Wk, np.float32)
    Wv = np.asarray(Wv, np.float32)
    Wo = np.asarray(Wo, np.float32)
    bq = np.asarray(bq, np.float32)
    bk = np.asarray(bk, np.float32)
    bv = np.asarray(bv, np.float32)
    bo = np.asarray(bo, np.float32)
    assert not np.any(bv), "kernel assumes bv == 0 (true for this problem)"

    nc = _get_prog()

    triu1 = np.triu(np.ones((128, 128), np.float32))
    triu2 = np.concatenate([triu1, triu1], axis=1)
    ident = np.eye(128, dtype=np.float32).astype(BF16)

    def tile_x(a):  # a: [S, D] -> [NSC, 128, IB, SCH], per-partition contiguous
        return a.reshape(NSC, SCH, IB, 128).transpose(0, 3, 2, 1).astype(BF16)

    def tile_w(w):  # w: [D, JS] -> [128, IB, JS]
        return w.reshape(IB, 128, JS).transpose(1, 0, 2).astype(BF16)

    xqs = [tile_x(qm[b]) for b in range(B)]
    xks = [tile_x(km[b]) for b in range(B)]
    xvs = [tile_x(vm[b]) for b in range(B)]

    in_maps = []
    for c in range(N_CORES):
        b, g = divmod(c, HPC)
        js = slice(g * JS, (g + 1) * JS)
        in_maps.append({
            "xq": xqs[b], "xk": xks[b], "xv": xvs[b],
            "wq": tile_w(Wq[:, js]),
            "wk": tile_w(Wk[:, js]),
            "wv": tile_w(Wv[:, js]),
            "wo": Wo[js, :].reshape(2, 128, D).transpose(1, 0, 2).astype(BF16),
            "bq": np.ascontiguousarray(bq[js].reshape(2, 128)),
            "bk": np.ascontiguousarray(bk[js].reshape(2, 128)),
            "triu2": triu2, "ident": ident,
        })

    res = bass_utils.run_bass_kernel_spmd(
        nc, in_maps, core_ids=list(range(N_CORES)),
        trace=TRACE, trace_cores=TRACE_CORES)
    LAST_RESULTS = res

    out = np.zeros((B, S, D), np.float32)
    for c in range(N_CORES):
        out[c // HPC] += res.results[c]["po"].astype(np.float32).T
    out += bo
    return out


# revision 39
# speedup vs baseline: 1.0895x; 1.0077x over previous
"""Trainium2 Bass kernel for nn_MultiHeadAttention_KT (causal linear attention).

Math (per batch b):
  q' = leaky((q*qm) @ Wq + bq); k' = leaky((k*km) @ Wk + bk); v' = (v*vm) @ Wv
  per head h (DEPTH=64):   S_t = sum_{s<=t} k_s v_s^T ; z_t = sum_{s<=t} k_s
                           attn_t = (q_t @ S_t) / (q_t . z_t)
  out = concat_heads(attn) @ Wo + bo
Sharding: 8 cores = 2 batches x 4 head-groups (4 heads / 256 cols each).
Host transposes + bf16-casts inputs (xq = (q*qm)^T etc.); host sums the 4
partial output projections per batch (po is this core's heads' Wo slice).

All matmul operands are bf16 (1 cyc/row on PE + fast weight load); PSUM
accumulation stays f32.  Chunked linear attention (chunk C=128):
  AT   = K Q^T (per chunk, [s,t] layout)      masked with triu (s<=t)
  num  = ATm^T V_aug + Q S_aug                (V_aug = [V | 1], S_aug = [S | z])
  attn = num[:, :64] * (1/num[:, 64])
  S_aug += K_chunk^T V_aug                    (delta matmul; f32 master state
                                               on DVE + bf16 mirror)

Schedule: the 512-col projection / output-projection matmul groups are
pumped one-at-a-time between attention-chunk stages so the PE array duty
stays high in every HAM window (keeps the 2.4 GHz clock gate open), DMA
overlaps compute, and the store tail is hidden.
"""

import os
import sys

sys.path.insert(0, "/opt/trn_rl_repo")

import ml_dtypes
import numpy as np

BF16 = np.dtype(ml_dtypes.bfloat16)

B, S, D, H = 2, 2048, 1024, 16
DEPTH = 64
N_CORES = 8
HPC = 4                 # heads per core
JS = HPC * DEPTH        # 256 projected columns per core
C = 128                 # attention chunk length
NCH = S // C            # 16 chunks
IB = D // 128           # 8 contraction blocks
SCH = 512               # projection s-chunk
NSC = S // SCH          # 4 projection chunks
JAUG = DEPTH + 1        # 65 (V augmented with ones column)
OSC = 512               # output projection s-chunk
CPO = OSC // C          # attention chunks per output chunk

MM_DTYPE = "bf16"       # informational (printed by test harness)
SIM_NO_PRELU = os.environ.get("KT_SIM_NO_PRELU") == "1"  # CoreSim lacks Prelu
TRACE = False           # set True from test harness to capture NTFF profile
TRACE_CORES = None
LAST_RESULTS = None     # BassKernelResults of the last kernel() call

_PROG = None


def _build():
    import concourse.bacc as bacc
    import concourse.mybir as mybir
    import concourse.tile as tile

    dt = mybir.dt
    f32 = dt.float32
    bf = dt.bfloat16
    AF = mybir.ActivationFunctionType
    Alu = mybir.AluOpType

    nc = bacc.Bacc("TRN2", target_bir_lowering=False, debug=False,
                   num_devices=N_CORES)

    # host pre-tiles x and weights so every DMA is per-partition contiguous
    xq = nc.dram_tensor("xq", [NSC, 128, IB, SCH], bf, kind="ExternalInput").ap()
    xk = nc.dram_tensor("xk", [NSC, 128, IB, SCH], bf, kind="ExternalInput").ap()
    xv = nc.dram_tensor("xv", [NSC, 128, IB, SCH], bf, kind="ExternalInput").ap()
    wq = nc.dram_tensor("wq", [128, IB, JS], bf, kind="ExternalInput").ap()
    wk = nc.dram_tensor("wk", [128, IB, JS], bf, kind="ExternalInput").ap()
    wv = nc.dram_tensor("wv", [128, IB, JS], bf, kind="ExternalInput").ap()
    wo = nc.dram_tensor("wo", [128, 2, D], bf, kind="ExternalInput").ap()
    bqd = nc.dram_tensor("bq", [2, 128], f32, kind="ExternalInput").ap()
    bkd = nc.dram_tensor("bk", [2, 128], f32, kind="ExternalInput").ap()
    triu2 = nc.dram_tensor("triu2", [128, 256], f32, kind="ExternalInput").ap()
    ident = nc.dram_tensor("ident", [128, 128], bf, kind="ExternalInput").ap()
    po = nc.dram_tensor("po", [D, S], bf, kind="ExternalOutput").ap()

    def mm(out, lhsT, rhs, **kw):
        nc.tensor.matmul(out, lhsT, rhs, **kw)

    with tile.TileContext(nc) as tc:
        with (
            tc.tile_pool(name="persist", bufs=1) as pp,
            tc.tile_pool(name="xin", bufs=3) as xpool,
            tc.tile_pool(name="work", bufs=4) as wk_pool,
            tc.tile_pool(name="outp", bufs=6) as opool,
            tc.tile_pool(name="psA", bufs=3, space="PSUM") as psA,
            tc.tile_pool(name="psB", bufs=5, space="PSUM") as psB,
        ):
            # ---- persistent tiles -------------------------------------------
            wq_sb = pp.tile([128, IB, JS], bf, tag="wq", name="wq_sb")
            wk_sb = pp.tile([128, IB, JS], bf, tag="wk", name="wk_sb")
            wv_sb = pp.tile([128, IB, JS], bf, tag="wv", name="wv_sb")
            wo_sb = pp.tile([128, 2, D], bf, tag="wo", name="wo_sb")
            bq_sb = pp.tile([128, 2], f32, tag="bq", name="bq_sb")
            bk_sb = pp.tile([128, 2], f32, tag="bk", name="bk_sb")
            triu_sb = pp.tile([128, 256], f32, tag="triu", name="triu_sb")
            ident_sb = pp.tile([128, 128], bf, tag="ident", name="ident_sb")

            qT_sb = [pp.tile([128, S], bf, tag=f"qT{jb}", name=f"qT{jb}") for jb in range(2)]
            kT_sb = [pp.tile([128, S], bf, tag=f"kT{jb}", name=f"kT{jb}") for jb in range(2)]
            aT_c = pp.tile([128, 2, S], bf, tag="aTc", name="aTc")
            vaug_sb = [pp.tile([128, HPC * JAUG], bf, tag=f"vaug{i}", name=f"vaug{i}")
                       for i in range(NCH)]
            # two heads per tile: head h at partitions (h%2)*64 .. +64
            saug_sb = [pp.tile([128, JAUG], f32, tag=f"saug{jb}", name=f"saug{jb}")
                       for jb in range(2)]
            saug_bf = [pp.tile([128, JAUG], bf, tag=f"saugb{jb}", name=f"saugb{jb}")
                       for jb in range(2)]
            attn2_sb = [pp.tile([128, 2 * DEPTH], bf, tag=f"attn2{jb}", name=f"attn2{jb}")
                        for jb in range(2)]

            # ---- initial loads: q path first so compute starts ASAP ---------
            x_tiles = {}

            def load_x(sc):
                xq_t = xpool.tile([128, IB, SCH], bf, tag="xq")
                xk_t = xpool.tile([128, IB, SCH], bf, tag="xk")
                xv_t = xpool.tile([128, IB, SCH], bf, tag="xv")
                nc.sync.dma_start(xq_t[:], xq[sc])
                nc.scalar.dma_start(xk_t[:], xk[sc])
                nc.sync.dma_start(xv_t[:], xv[sc])
                x_tiles[sc] = (xq_t, xk_t, xv_t)

            # Startup is per-queue DMA-bandwidth bound (~3.5us per 0.5MB on
            # one queue), so the chunk-0 critical path is spread across
            # THREE queues (sync + scalar HWDGE, gpsimd SWDGE) in the exact
            # granularity the first matmul units consume: x in 256-col
            # halves, weights in per-jb 128-col halves.
            HA = SCH // 2
            xq_t0 = xpool.tile([128, IB, SCH], bf, tag="xq")
            xk_t0 = xpool.tile([128, IB, SCH], bf, tag="xk")
            xv_t0 = xpool.tile([128, IB, SCH], bf, tag="xv")
            nc.sync.dma_start(xq_t0[:, :, 0:HA], xq[0][:, :, 0:HA])
            nc.scalar.dma_start(wq_sb[:, :, 0:128], wq[:, :, 0:128])
            nc.gpsimd.dma_start(xk_t0[:, :, 0:HA], xk[0][:, :, 0:HA])
            nc.scalar.dma_start(wq_sb[:, :, 128:JS], wq[:, :, 128:JS])
            nc.scalar.dma_start(bq_sb[:], bqd.rearrange("jb p -> p jb"))
            nc.scalar.dma_start(bk_sb[:], bkd.rearrange("jb p -> p jb"))
            nc.sync.dma_start(xq_t0[:, :, HA:SCH], xq[0][:, :, HA:SCH])
            nc.scalar.dma_start(wk_sb[:, :, 0:128], wk[:, :, 0:128])
            nc.gpsimd.dma_start(xk_t0[:, :, HA:SCH], xk[0][:, :, HA:SCH])
            nc.scalar.dma_start(wk_sb[:, :, 128:JS], wk[:, :, 128:JS])
            nc.sync.dma_start(xv_t0[:, :, 0:HA], xv[0][:, :, 0:HA])
            nc.scalar.dma_start(xv_t0[:, :, HA:SCH], xv[0][:, :, HA:SCH])
            nc.gpsimd.dma_start(wv_sb[:], wv)
            nc.gpsimd.dma_start(triu_sb[:], triu2)
            nc.gpsimd.dma_start(ident_sb[:], ident)
            nc.gpsimd.dma_start(wo_sb[:], wo)
            x_tiles[0] = (xq_t0, xk_t0, xv_t0)
            load_x(1)

            # ---- pumpable work units (one PSUM group each) ------------------
            def unit_qk(which, sc, jb):
                s0 = sc * SCH
                x_t = x_tiles[sc][0 if which == "q" else 1]
                w_sb = wq_sb if which == "q" else wk_sb
                b_sb = bq_sb if which == "q" else bk_sb
                dst = qT_sb if which == "q" else kT_sb
                ps = psA.tile([128, SCH], f32, tag="A")
                for ib in range(IB):
                    mm(ps[:], w_sb[:, ib, jb * 128:(jb + 1) * 128],
                       x_t[:, ib, :],
                       start=(ib == 0), stop=(ib == IB - 1))
                nc.scalar.activation(
                    dst[jb][:, s0:s0 + SCH], ps[:],
                    AF.Identity if SIM_NO_PRELU else AF.Prelu,
                    bias=b_sb[:, jb:jb + 1], scale=1.0, alpha=0.1)

            def unit_v(sc, ss):
                x_t = x_tiles[sc][2]
                ps = psA.tile([128, JS], f32, tag="A")
                for ib in range(IB):
                    mm(ps[:], x_t[:, ib, ss * 128:(ss + 1) * 128],
                       wv_sb[:, ib, :],
                       start=(ib == 0), stop=(ib == IB - 1))
                vt = vaug_sb[sc * (SCH // 128) + ss]
                vt_r = vt[:].rearrange("p (h e) -> p h e", h=HPC)
                nc.scalar.activation(
                    vt_r[:, :, 0:DEPTH],
                    ps[:].rearrange("p (h e) -> p h e", h=HPC), AF.Copy)
                nc.vector.memset(vt_r[:, :, DEPTH:JAUG], 1.0)

            def unit_qk_half(which, sc, jb, half):
                s0 = sc * SCH + half * (SCH // 2)
                c0 = half * (SCH // 2)
                x_t = x_tiles[sc][0 if which == "q" else 1]
                w_sb = wq_sb if which == "q" else wk_sb
                b_sb = bq_sb if which == "q" else bk_sb
                dst = qT_sb if which == "q" else kT_sb
                ps = psA.tile([128, SCH // 2], f32, tag="A")
                for ib in range(IB):
                    mm(ps[:], w_sb[:, ib, jb * 128:(jb + 1) * 128],
                       x_t[:, ib, c0:c0 + SCH // 2],
                       start=(ib == 0), stop=(ib == IB - 1))
                nc.scalar.activation(
                    dst[jb][:, s0:s0 + SCH // 2], ps[:],
                    AF.Identity if SIM_NO_PRELU else AF.Prelu,
                    bias=b_sb[:, jb:jb + 1], scale=1.0, alpha=0.1)

            def proj_units(sc):
                u = []
                for jb in range(2):
                    u.append(lambda jb=jb: unit_qk("q", sc, jb))
                    u.append(lambda jb=jb: unit_qk("k", sc, jb))
                for ss in range(SCH // 128):
                    u.append(lambda ss=ss: unit_v(sc, ss))
                return u

            def proj_units0():
                # chunk-0 variant: q/k at half-chunk granularity, ordered to
                # match the staggered arrival of the split input DMAs
                u = []
                for half in range(2):
                    for which in ("q", "k"):
                        for jb in range(2):
                            u.append(lambda w=which, jb=jb, h=half:
                                     unit_qk_half(w, 0, jb, h))
                for ss in range(SCH // 128):
                    u.append(lambda ss=ss: unit_v(0, ss))
                return u

            po_r = po.rearrange("(ob p) s -> ob p s", p=128)

            def unit_p3(o0, osc, ob):
                ps = psA.tile([128, OSC], f32, tag="A")
                for jb in range(2):
                    mm(ps[:, 0:osc], wo_sb[:, jb, ob * 128:(ob + 1) * 128],
                       aT_c[:, jb, o0:o0 + osc],
                       start=(jb == 0), stop=(jb == 1))
                ot = opool.tile([128, OSC], bf, tag="ot")
                if ob % 2 == 0:
                    nc.vector.tensor_copy(ot[:, 0:osc], ps[:, 0:osc])
                else:
                    nc.scalar.activation(ot[:, 0:osc], ps[:, 0:osc], AF.Copy)
                q_eng = nc.sync if ob % 2 == 0 else nc.scalar
                q_eng.dma_start(po_r[ob, :, o0:o0 + osc], ot[:, 0:osc])

            def p3_units(o0, osc=OSC):
                return [lambda ob=ob: unit_p3(o0, osc, ob)
                        for ob in range(D // 128)]

            pending = []

            def pump():
                if pending:
                    u = pending.pop(0)
                    if u is not None:
                        u()

            # ---- attention chunk (pumps a work unit between stages) ---------
            def chunk(ci):
                scol = ci * C
                if ci > 0:
                    for jb in range(2):
                        nc.vector.tensor_copy(saug_bf[jb][:], saug_sb[jb][:])

                # stage 1: K transposes (both heads in one op) + scores
                knats = []
                atm = []
                for jb in range(2):
                    knat_ps = psB.tile([128, 2 * DEPTH], bf, tag="B")
                    nc.tensor.transpose(knat_ps[:],
                                        kT_sb[jb][:, scol:scol + C],
                                        ident_sb[:])
                    knat = wk_pool.tile([128, 2 * DEPTH], bf, tag="knat")
                    nc.vector.tensor_copy(knat[:], knat_ps[:])
                    knats.append(knat)
                    am = wk_pool.tile([128, 2 * C], bf, tag="atm")
                    for hh in range(2):
                        jo = hh * DEPTH
                        at = psA.tile([128, C], f32, tag="A")
                        mm(at[:], kT_sb[jb][jo:jo + DEPTH, scol:scol + C],
                           qT_sb[jb][jo:jo + DEPTH, scol:scol + C],
                           start=True, stop=True)
                        nc.vector.tensor_tensor(am[:, hh * C:(hh + 1) * C],
                                                at[:], triu_sb[:, 0:C],
                                                op=Alu.mult)
                    atm.append(am)
                pump()

                # stage 2: numerators + attn, per jb
                for jb in range(2):
                    for hh in range(2):
                        jo = hh * DEPTH
                        h = jb * 2 + hh
                        vt = vaug_sb[ci][:, h * JAUG:(h + 1) * JAUG]
                        nump = psB.tile([128, JAUG], f32, tag="B")
                        mm(nump[:], atm[jb][:, hh * C:(hh + 1) * C], vt,
                           start=True, stop=(ci == 0))
                        if ci > 0:
                            mm(nump[:], qT_sb[jb][jo:jo + DEPTH, scol:scol + C],
                               saug_bf[jb][jo:jo + DEPTH, :],
                               start=False, stop=True)
                        recip = wk_pool.tile([128, 1], f32, tag="recip")
                        nc.vector.reciprocal(recip[:], nump[:, DEPTH:JAUG])
                        dstap = attn2_sb[jb][:, hh * DEPTH:(hh + 1) * DEPTH]
                        if jb == 0:
                            nc.vector.tensor_scalar_mul(
                                dstap, nump[:, 0:DEPTH], recip[:])
                        else:
                            nc.scalar.activation(dstap, nump[:, 0:DEPTH],
                                                 AF.Copy, scale=recip[:])
                    pump()

                # stage 3: state update S_aug += K^T V_aug
                if ci < NCH - 1:
                    for jb in range(2):
                        d_ps = psB.tile([128, JAUG], f32, tag="B")
                        for hh in range(2):
                            jo = hh * DEPTH
                            h = jb * 2 + hh
                            vt = vaug_sb[ci][:, h * JAUG:(h + 1) * JAUG]
                            mm(d_ps[jo:jo + DEPTH, :],
                               knats[jb][:, jo:jo + DEPTH],
                               vt, start=True, stop=True)
                        if ci == 0:
                            nc.vector.tensor_copy(saug_sb[jb][:], d_ps[:])
                        else:
                            nc.vector.tensor_add(saug_sb[jb][:],
                                                 saug_sb[jb][:], d_ps[:])

                # stage 4: transpose attn -> aT columns
                for jb in range(2):
                    at2_ps = psB.tile([128, C], bf, tag="B")
                    nc.tensor.transpose(at2_ps[:], attn2_sb[jb][:], ident_sb[:])
                    nc.scalar.activation(aT_c[:, jb, scol:scol + C],
                                         at2_ps[:], AF.Copy)
                pump()

            # ---- schedule ---------------------------------------------------
            # Groups 1-2 need >=4 pump units per chunk to hold the PE clock
            # gate open (measured: 3.5/chunk already re-throttles), so they
            # keep the full 8 projection + 8 output-projection units; group 3
            # gets what remains and the last 512 columns run as one dense
            # block after the final chunk.
            for u in proj_units0():
                u()
            for g in range(NSC):
                if g + 2 < NSC:
                    load_x(g + 2)
                pending = []
                a = proj_units(g + 1) if g + 1 < NSC else []
                if g == NSC - 1:
                    pending = []
                    for u in p3_units((g - 1) * OSC):
                        pending.append(u)
                        pending.append(None)
                else:
                    b = p3_units((g - 1) * OSC) if g >= 1 else [None] * 8
                    while a or b:
                        if a:
                            pending.append(a.pop(0))
                        if b:
                            pending.append(b.pop(0))
                for t in range(CPO):
                    chunk(CPO * g + t)
                while pending:
                    pump()
            # tail: the last group's full 512 output columns as one dense
            # block of 512-col matmuls (back-to-back keeps the clock gate
            # open; copies and stores trail on vector/scalar + both queues)
            for u in p3_units((NSC - 1) * OSC):
                u()

    nc.compile()
    return nc


def _get_prog():
    global _PROG
    if _PROG is None:
        _PROG = _build()
    return _PROG


def kernel(q, k, v, query_mask, key_mask, value_mask,
           Wq, bq, Wk, bk, Wv, bv, Wo, bo):
    global LAST_RESULTS
    from concourse import bass_utils

    q = np.asarray(q, np.float32)
    k = np.asarray(k, np.float32)
    v = np.asarray(v, np.float32)
    qm = q * np.asarray(query_mask, np.float32)
    km = k * np.asarray(key_mask, np.float32)
    vm = v * np.asarray(value_mask, np.float32)
    Wq = np.asarray(Wq, np.float32)
    Wk = np.asarray(Wk, np.float32)
    Wv = np.asarray(Wv, np.float32)
    Wo = np.asarray(Wo, np.float32)
    bq = np.asarray(bq, np.float32)
    bk = np.asarray(bk, np.float32)
    bv = np.asarray(bv, np.float32)
    bo = np.asarray(bo, np.float32)
    assert not np.any(bv), "kernel assumes bv == 0 (true for this problem)"

    nc = _get_prog()

    triu1 = np.triu(np.ones((128, 128), np.float32))
    triu2 = np.concatenate([triu1, triu1], axis=1)
    ident = np.eye(128, dtype=np.float32).astype(BF16)

    def tile_x(a):  # a: [S, D] -> [NSC, 128, IB, SCH], per-partition contiguous
        return a.reshape(NSC, SCH, IB, 128).transpose(0, 3, 2, 1).astype(BF16)

    def tile_w(w):  # w: [D, JS] -> [128, IB, JS]
        return w.reshape(IB, 128, JS).transpose(1, 0, 2).astype(BF16)

    xqs = [tile_x(qm[b]) for b in range(B)]
    xks = [tile_x(km[b]) for b in range(B)]
    xvs = [tile_x(vm[b]) for b in range(B)]

    in_maps = []
    for c in range(N_CORES):
        b, g = divmod(c, HPC)
        js = slice(g * JS, (g + 1) * JS)
        in_maps.append({
            "xq": xqs[b], "xk": xks[b], "xv": xvs[b],
            "wq": tile_w(Wq[:, js]),
            "wk": tile_w(Wk[:, js]),
            "wv": tile_w(Wv[:, js]),
            "wo": Wo[js, :].reshape(2, 128, D).transpose(1, 0, 2).astype(BF16),
            "bq": np.ascontiguousarray(bq[js].reshape(2, 128)),
            "bk": np.ascontiguousarray(bk[js].reshape(2, 128)),
            "triu2": triu2, "ident": ident,
        })

    res = bass_utils.run_bass_kernel_spmd(
        nc, in_maps, core_ids=list(range(N_CORES)),
        trace=TRACE, trace_cores=TRACE_CORES)
    LAST_RESULTS = res

    out = np.zeros((B, S, D), np.float32)
    for c in range(N_CORES):
        out[c // HPC] += res.results[c]["po"].astype(np.float32).T
    out += bo
    return out


# revision 40
# speedup vs baseline: 1.2277x; 1.1268x over previous
"""Trainium2 Bass kernel for nn_MultiHeadAttention_KT (causal linear attention).

Math (per batch b):
  q' = leaky((q*qm) @ Wq + bq); k' = leaky((k*km) @ Wk + bk); v' = (v*vm) @ Wv
  per head h (DEPTH=64):   S_t = sum_{s<=t} k_s v_s^T ; z_t = sum_{s<=t} k_s
                           attn_t = (q_t @ S_t) / (q_t . z_t)
  out = concat_heads(attn) @ Wo + bo
Sharding: 8 cores = 2 batches x 4 head-groups (4 heads / 256 cols each).
Host transposes + bf16-casts inputs (xq = (q*qm)^T etc.); host sums the 4
partial output projections per batch (po is this core's heads' Wo slice).

All matmul operands are bf16 (1 cyc/row on PE + fast weight load); PSUM
accumulation stays f32.  Chunked linear attention (chunk C=128):
  AT   = K Q^T (per chunk, [s,t] layout)      masked with triu (s<=t)
  num  = ATm^T V_aug + Q S_aug                (V_aug = [V | 1], S_aug = [S | z])
  attn = num[:, :64] * (1/num[:, 64])
  S_aug += K_chunk^T V_aug                    (delta matmul; f32 master state
                                               on DVE + bf16 mirror)

Schedule: the 512-col projection / output-projection matmul groups are
pumped one-at-a-time between attention-chunk stages so the PE array duty
stays high in every HAM window (keeps the 2.4 GHz clock gate open), DMA
overlaps compute, and the store tail is hidden.
"""

import os
import sys

sys.path.insert(0, "/opt/trn_rl_repo")

import ml_dtypes
import numpy as np

BF16 = np.dtype(ml_dtypes.bfloat16)

B, S, D, H = 2, 2048, 1024, 16
DEPTH = 64
N_CORES = 8
HPC = 4                 # heads per core
JS = HPC * DEPTH        # 256 projected columns per core
C = 128                 # attention chunk length
NCH = S // C            # 16 chunks
IB = D // 128           # 8 contraction blocks
SCH = 512               # projection s-chunk
NSC = S // SCH          # 4 projection chunks
JAUG = DEPTH + 1        # 65 (V augmented with ones column)
OSC = 512               # output projection s-chunk
CPO = OSC // C          # attention chunks per output chunk

MM_DTYPE = "bf16"       # informational (printed by test harness)
SIM_NO_PRELU = os.environ.get("KT_SIM_NO_PRELU") == "1"  # CoreSim lacks Prelu
TRACE = False           # set True from test harness to capture NTFF profile
TRACE_CORES = None
LAST_RESULTS = None     # BassKernelResults of the last kernel() call

_PROG = None


def _build():
    import concourse.bacc as bacc
    import concourse.mybir as mybir
    import concourse.tile as tile

    dt = mybir.dt
    f32 = dt.float32
    bf = dt.bfloat16
    AF = mybir.ActivationFunctionType
    Alu = mybir.AluOpType

    nc = bacc.Bacc("TRN2", target_bir_lowering=False, debug=False,
                   num_devices=N_CORES)

    # host pre-tiles x and weights so every DMA is per-partition contiguous
    xq = nc.dram_tensor("xq", [NSC, 128, IB, SCH], bf, kind="ExternalInput").ap()
    xk = nc.dram_tensor("xk", [NSC, 128, IB, SCH], bf, kind="ExternalInput").ap()
    xv = nc.dram_tensor("xv", [NSC, 128, IB, SCH], bf, kind="ExternalInput").ap()
    wq = nc.dram_tensor("wq", [128, IB, JS], bf, kind="ExternalInput").ap()
    wk = nc.dram_tensor("wk", [128, IB, JS], bf, kind="ExternalInput").ap()
    wv = nc.dram_tensor("wv", [128, IB, JS], bf, kind="ExternalInput").ap()
    wo = nc.dram_tensor("wo", [128, 2, D], bf, kind="ExternalInput").ap()
    bqd = nc.dram_tensor("bq", [2, 128], f32, kind="ExternalInput").ap()
    bkd = nc.dram_tensor("bk", [2, 128], f32, kind="ExternalInput").ap()
    triu2 = nc.dram_tensor("triu2", [128, 256], f32, kind="ExternalInput").ap()
    ident = nc.dram_tensor("ident", [128, 128], bf, kind="ExternalInput").ap()
    po = nc.dram_tensor("po", [D, S], bf, kind="ExternalOutput").ap()

    def mm(out, lhsT, rhs, **kw):
        nc.tensor.matmul(out, lhsT, rhs, **kw)

    with tile.TileContext(nc) as tc:
        with (
            tc.tile_pool(name="persist", bufs=1) as pp,
            tc.tile_pool(name="xin", bufs=3) as xpool,
            tc.tile_pool(name="work", bufs=4) as wk_pool,
            tc.tile_pool(name="outp", bufs=6) as opool,
            tc.tile_pool(name="psA", bufs=3, space="PSUM") as psA,
            tc.tile_pool(name="psB", bufs=5, space="PSUM") as psB,
        ):
            # ---- persistent tiles -------------------------------------------
            wq_sb = pp.tile([128, IB, JS], bf, tag="wq", name="wq_sb")
            wk_sb = pp.tile([128, IB, JS], bf, tag="wk", name="wk_sb")
            wv_sb = pp.tile([128, IB, JS], bf, tag="wv", name="wv_sb")
            wo_sb = pp.tile([128, 2, D], bf, tag="wo", name="wo_sb")
            bq_sb = pp.tile([128, 2], f32, tag="bq", name="bq_sb")
            bk_sb = pp.tile([128, 2], f32, tag="bk", name="bk_sb")
            triu_sb = pp.tile([128, 256], f32, tag="triu", name="triu_sb")
            ident_sb = pp.tile([128, 128], bf, tag="ident", name="ident_sb")

            qT_sb = [pp.tile([128, S], bf, tag=f"qT{jb}", name=f"qT{jb}") for jb in range(2)]
            kT_sb = [pp.tile([128, S], bf, tag=f"kT{jb}", name=f"kT{jb}") for jb in range(2)]
            aT_c = pp.tile([128, 2, S], bf, tag="aTc", name="aTc")
            vaug_sb = [pp.tile([128, HPC * JAUG], bf, tag=f"vaug{i}", name=f"vaug{i}")
                       for i in range(NCH)]
            # two heads per tile: head h at partitions (h%2)*64 .. +64
            saug_sb = [pp.tile([128, JAUG], f32, tag=f"saug{jb}", name=f"saug{jb}")
                       for jb in range(2)]
            saug_bf = [pp.tile([128, JAUG], bf, tag=f"saugb{jb}", name=f"saugb{jb}")
                       for jb in range(2)]
            attn2_sb = [pp.tile([128, 2 * DEPTH], bf, tag=f"attn2{jb}", name=f"attn2{jb}")
                        for jb in range(2)]

            # ---- initial loads: q path first so compute starts ASAP ---------
            x_tiles = {}

            def load_x(sc):
                xq_t = xpool.tile([128, IB, SCH], bf, tag="xq")
                xk_t = xpool.tile([128, IB, SCH], bf, tag="xk")
                xv_t = xpool.tile([128, IB, SCH], bf, tag="xv")
                nc.sync.dma_start(xq_t[:], xq[sc])
                nc.scalar.dma_start(xk_t[:], xk[sc])
                nc.sync.dma_start(xv_t[:], xv[sc])
                x_tiles[sc] = (xq_t, xk_t, xv_t)

            # chunk-0 inputs arrive in 256-col halves so the first projection
            # matmuls can start as soon as the first half lands.  Only the
            # two HWDGE queues (sync/scalar) are used: the gpsimd SWDGE ring
            # generates descriptors too slowly to help at startup.
            HA = SCH // 2
            xq_t0 = xpool.tile([128, IB, SCH], bf, tag="xq")
            xk_t0 = xpool.tile([128, IB, SCH], bf, tag="xk")
            xv_t0 = xpool.tile([128, IB, SCH], bf, tag="xv")
            nc.sync.dma_start(xq_t0[:, :, 0:HA], xq[0][:, :, 0:HA])
            nc.scalar.dma_start(wq_sb[:], wq)
            nc.scalar.dma_start(bq_sb[:], bqd.rearrange("jb p -> p jb"))
            nc.scalar.dma_start(bk_sb[:], bkd.rearrange("jb p -> p jb"))
            nc.sync.dma_start(xq_t0[:, :, HA:SCH], xq[0][:, :, HA:SCH])
            nc.scalar.dma_start(xk_t0[:, :, 0:HA], xk[0][:, :, 0:HA])
            nc.scalar.dma_start(wk_sb[:], wk)
            nc.sync.dma_start(xv_t0[:, :, 0:HA], xv[0][:, :, 0:HA])
            nc.scalar.dma_start(xk_t0[:, :, HA:SCH], xk[0][:, :, HA:SCH])
            nc.sync.dma_start(xv_t0[:, :, HA:SCH], xv[0][:, :, HA:SCH])
            nc.sync.dma_start(wv_sb[:], wv)
            nc.scalar.dma_start(triu_sb[:], triu2)
            nc.scalar.dma_start(ident_sb[:], ident)
            nc.scalar.dma_start(wo_sb[:], wo)
            x_tiles[0] = (xq_t0, xk_t0, xv_t0)
            load_x(1)

            # ---- pumpable work units (one PSUM group each) ------------------
            def unit_qk(which, sc, jb):
                s0 = sc * SCH
                x_t = x_tiles[sc][0 if which == "q" else 1]
                w_sb = wq_sb if which == "q" else wk_sb
                b_sb = bq_sb if which == "q" else bk_sb
                dst = qT_sb if which == "q" else kT_sb
                ps = psA.tile([128, SCH], f32, tag="A")
                for ib in range(IB):
                    mm(ps[:], w_sb[:, ib, jb * 128:(jb + 1) * 128],
                       x_t[:, ib, :],
                       start=(ib == 0), stop=(ib == IB - 1))
                nc.scalar.activation(
                    dst[jb][:, s0:s0 + SCH], ps[:],
                    AF.Identity if SIM_NO_PRELU else AF.Prelu,
                    bias=b_sb[:, jb:jb + 1], scale=1.0, alpha=0.1)

            def unit_v(sc, ss):
                x_t = x_tiles[sc][2]
                ps = psA.tile([128, JS], f32, tag="A")
                for ib in range(IB):
                    mm(ps[:], x_t[:, ib, ss * 128:(ss + 1) * 128],
                       wv_sb[:, ib, :],
                       start=(ib == 0), stop=(ib == IB - 1))
                vt = vaug_sb[sc * (SCH // 128) + ss]
                vt_r = vt[:].rearrange("p (h e) -> p h e", h=HPC)
                nc.scalar.activation(
                    vt_r[:, :, 0:DEPTH],
                    ps[:].rearrange("p (h e) -> p h e", h=HPC), AF.Copy)
                nc.vector.memset(vt_r[:, :, DEPTH:JAUG], 1.0)

            def unit_qk_half(which, sc, jb, half):
                s0 = sc * SCH + half * (SCH // 2)
                c0 = half * (SCH // 2)
                x_t = x_tiles[sc][0 if which == "q" else 1]
                w_sb = wq_sb if which == "q" else wk_sb
                b_sb = bq_sb if which == "q" else bk_sb
                dst = qT_sb if which == "q" else kT_sb
                ps = psA.tile([128, SCH // 2], f32, tag="A")
                for ib in range(IB):
                    mm(ps[:], w_sb[:, ib, jb * 128:(jb + 1) * 128],
                       x_t[:, ib, c0:c0 + SCH // 2],
                       start=(ib == 0), stop=(ib == IB - 1))
                nc.scalar.activation(
                    dst[jb][:, s0:s0 + SCH // 2], ps[:],
                    AF.Identity if SIM_NO_PRELU else AF.Prelu,
                    bias=b_sb[:, jb:jb + 1], scale=1.0, alpha=0.1)

            def proj_units(sc):
                u = []
                for jb in range(2):
                    u.append(lambda jb=jb: unit_qk("q", sc, jb))
                    u.append(lambda jb=jb: unit_qk("k", sc, jb))
                for ss in range(SCH // 128):
                    u.append(lambda ss=ss: unit_v(sc, ss))
                return u

            def proj_units0():
                # chunk-0 variant: q/k at half-chunk granularity, ordered to
                # match the staggered arrival of the split input DMAs
                u = []
                for half in range(2):
                    for which in ("q", "k"):
                        for jb in range(2):
                            u.append(lambda w=which, jb=jb, h=half:
                                     unit_qk_half(w, 0, jb, h))
                for ss in range(SCH // 128):
                    u.append(lambda ss=ss: unit_v(0, ss))
                return u

            po_r = po.rearrange("(ob p) s -> ob p s", p=128)

            def unit_p3(o0, osc, ob):
                ps = psA.tile([128, OSC], f32, tag="A")
                for jb in range(2):
                    mm(ps[:, 0:osc], wo_sb[:, jb, ob * 128:(ob + 1) * 128],
                       aT_c[:, jb, o0:o0 + osc],
                       start=(jb == 0), stop=(jb == 1))
                ot = opool.tile([128, OSC], bf, tag="ot")
                if ob % 2 == 0:
                    nc.vector.tensor_copy(ot[:, 0:osc], ps[:, 0:osc])
                else:
                    nc.scalar.activation(ot[:, 0:osc], ps[:, 0:osc], AF.Copy)
                q_eng = nc.sync if ob % 2 == 0 else nc.scalar
                q_eng.dma_start(po_r[ob, :, o0:o0 + osc], ot[:, 0:osc])

            def p3_units(o0, osc=OSC):
                return [lambda ob=ob: unit_p3(o0, osc, ob)
                        for ob in range(D // 128)]

            pending = []

            def pump():
                if pending:
                    u = pending.pop(0)
                    if u is not None:
                        u()

            # ---- attention chunk (pumps a work unit between stages) ---------
            def chunk(ci):
                scol = ci * C
                if ci > 0:
                    for jb in range(2):
                        nc.vector.tensor_copy(saug_bf[jb][:], saug_sb[jb][:])

                # stage 1: K transposes (both heads in one op) + scores
                knats = []
                atm = []
                for jb in range(2):
                    knat_ps = psB.tile([128, 2 * DEPTH], bf, tag="B")
                    nc.tensor.transpose(knat_ps[:],
                                        kT_sb[jb][:, scol:scol + C],
                                        ident_sb[:])
                    knat = wk_pool.tile([128, 2 * DEPTH], bf, tag="knat")
                    nc.vector.tensor_copy(knat[:], knat_ps[:])
                    knats.append(knat)
                    am = wk_pool.tile([128, 2 * C], bf, tag="atm")
                    for hh in range(2):
                        jo = hh * DEPTH
                        at = psA.tile([128, C], f32, tag="A")
                        mm(at[:], kT_sb[jb][jo:jo + DEPTH, scol:scol + C],
                           qT_sb[jb][jo:jo + DEPTH, scol:scol + C],
                           start=True, stop=True)
                        nc.vector.tensor_tensor(am[:, hh * C:(hh + 1) * C],
                                                at[:], triu_sb[:, 0:C],
                                                op=Alu.mult)
                    atm.append(am)
                pump()

                # stage 2: numerators + attn, per jb
                for jb in range(2):
                    for hh in range(2):
                        jo = hh * DEPTH
                        h = jb * 2 + hh
                        vt = vaug_sb[ci][:, h * JAUG:(h + 1) * JAUG]
                        nump = psB.tile([128, JAUG], f32, tag="B")
                        mm(nump[:], atm[jb][:, hh * C:(hh + 1) * C], vt,
                           start=True, stop=(ci == 0))
                        if ci > 0:
                            mm(nump[:], qT_sb[jb][jo:jo + DEPTH, scol:scol + C],
                               saug_bf[jb][jo:jo + DEPTH, :],
                               start=False, stop=True)
                        recip = wk_pool.tile([128, 1], f32, tag="recip")
                        nc.vector.reciprocal(recip[:], nump[:, DEPTH:JAUG])
                        dstap = attn2_sb[jb][:, hh * DEPTH:(hh + 1) * DEPTH]
                        if jb == 0:
                            nc.vector.tensor_scalar_mul(
                                dstap, nump[:, 0:DEPTH], recip[:])
                        else:
                            nc.scalar.activation(dstap, nump[:, 0:DEPTH],
                                                 AF.Copy, scale=recip[:])
                    pump()

                # stage 3: state update S_aug += K^T V_aug
                if ci < NCH - 1:
                    for jb in range(2):
                        d_ps = psB.tile([128, JAUG], f32, tag="B")
                        for hh in range(2):
                            jo = hh * DEPTH
                            h = jb * 2 + hh
                            vt = vaug_sb[ci][:, h * JAUG:(h + 1) * JAUG]
                            mm(d_ps[jo:jo + DEPTH, :],
                               knats[jb][:, jo:jo + DEPTH],
                               vt, start=True, stop=True)
                        if ci == 0:
                            nc.vector.tensor_copy(saug_sb[jb][:], d_ps[:])
                        else:
                            nc.vector.tensor_add(saug_sb[jb][:],
                                                 saug_sb[jb][:], d_ps[:])

                # stage 4: transpose attn -> aT columns
                for jb in range(2):
                    at2_ps = psB.tile([128, C], bf, tag="B")
                    nc.tensor.transpose(at2_ps[:], attn2_sb[jb][:], ident_sb[:])
                    nc.scalar.activation(aT_c[:, jb, scol:scol + C],
                                         at2_ps[:], AF.Copy)
                pump()

            # ---- schedule ---------------------------------------------------
            # Groups 1-2 need >=4 pump units per chunk to hold the PE clock
            # gate open (measured: 3.5/chunk already re-throttles), so they
            # keep the full 8 projection + 8 output-projection units; group 3
            # gets what remains and the last 512 columns run as one dense
            # block after the final chunk.
            for u in proj_units0():
                u()
            for g in range(NSC):
                if g + 2 < NSC:
                    load_x(g + 2)
                pending = []
                a = proj_units(g + 1) if g + 1 < NSC else []
                if g == NSC - 1:
                    pending = []
                    for u in p3_units((g - 1) * OSC):
                        pending.append(u)
                        pending.append(None)
                else:
                    b = p3_units((g - 1) * OSC) if g >= 1 else [None] * 8
                    while a or b:
                        if a:
                            pending.append(a.pop(0))
                        if b:
                            pending.append(b.pop(0))
                for t in range(CPO):
                    chunk(CPO * g + t)
                while pending:
                    pump()
            # tail: the last group's full 512 output columns as one dense
            # block of 512-col matmuls (back-to-back keeps the clock gate
            # open; copies and stores trail on vector/scalar + both queues)
            for u in p3_units((NSC - 1) * OSC):
                u()

    nc.compile()
    return nc


def _get_prog():
    global _PROG
    if _PROG is None:
        _PROG = _build()
    return _PROG


def kernel(q, k, v, query_mask, key_mask, value_mask,
           Wq, bq, Wk, bk, Wv, bv, Wo, bo):
    global LAST_RESULTS
    from concourse import bass_utils

    q = np.asarray(q, np.float32)
    k = np.asarray(k, np.float32)
    v = np.asarray(v, np.float32)
    qm = q * np.asarray(query_mask, np.float32)
    km = k * np.asarray(key_mask, np.float32)
    vm = v * np.asarray(value_mask, np.float32)
    Wq = np.asarray(Wq, np.float32)
    Wk = np.asarray(Wk, np.float32)
    Wv = np.asarray(Wv, np.float32)
    Wo = np.asarray(Wo, np.float32)
    bq = np.asarray(bq, np.float32)
    bk = np.asarray(bk, np.float32)
    bv = np.asarray(bv, np.float32)
    bo = np.asarray(bo, np.float32)
    assert not np.any(bv), "kernel assumes bv == 0 (true for this problem)"

    nc = _get_prog()

    triu1 = np.triu(np.ones((128, 128), np.float32))
    triu2 = np.concatenate([triu1, triu1], axis=1)
    ident = np.eye(128, dtype=np.float32).astype(BF16)

    def tile_x(a):  # a: [S, D] -> [NSC, 128, IB, SCH], per-partition contiguous
        return a.reshape(NSC, SCH, IB, 128).transpose(0, 3, 2, 1).astype(BF16)

    def tile_w(w):  # w: [D, JS] -> [128, IB, JS]
        return w.reshape(IB, 128, JS).transpose(1, 0, 2).astype(BF16)

    xqs = [tile_x(qm[b]) for b in range(B)]
    xks = [tile_x(km[b]) for b in range(B)]
    xvs = [tile_x(vm[b]) for b in range(B)]

    in_maps = []
    for c in range(N_CORES):
        b, g = divmod(c, HPC)
        js = slice(g * JS, (g + 1) * JS)
        in_maps.append({
            "xq": xqs[b], "xk": xks[b], "xv": xvs[b],
            "wq": tile_w(Wq[:, js]),
            "wk": tile_w(Wk[:, js]),
            "wv": tile_w(Wv[:, js]),
            "wo": Wo[js, :].reshape(2, 128, D).transpose(1, 0, 2).astype(BF16),
            "bq": np.ascontiguousarray(bq[js].reshape(2, 128)),
            "bk": np.ascontiguousarray(bk[js].reshape(2, 128)),
            "triu2": triu2, "ident": ident,
        })

    res = bass_utils.run_bass_kernel_spmd(
        nc, in_maps, core_ids=list(range(N_CORES)),
        trace=TRACE, trace_cores=TRACE_CORES)
    LAST_RESULTS = res

    out = np.zeros((B, S, D), np.float32)
    for c in range(N_CORES):
        out[c // HPC] += res.results[c]["po"].astype(np.float32).T
    out += bo
    return out
